# revision 6
# baseline (speedup 1.0000x reference)
"""Trainium2 Bass kernel for GQA attention (B=2,S=2048,D=2048,H=16,KV=4,HD=128)
with RoPE + causal mask, sharded over 8 NeuronCores:
  2-way data parallel over batch x 4-way tensor parallel over KV groups.

Mixed precision, chosen so the error-critical short-softmax rows stay in
higher precision while the bulk of the FLOPs run double-pumped fp8:
  - rows 0..511 (chunk 0): bf16 operands everywhere (proj, scores, probs,
    AV, out-proj), fp32 PSUM accumulation.  Short rows have little error
    averaging, bf16 keeps them at ~3e-3.
  - rows 512..2047 (chunks 1-3): fp8e4 (e4m3) operands with
    MatmulPerfMode.DoubleRow (2 contraction rows/cycle).  Softmax rows here
    average >=512 terms so fp8 noise stays ~6e-3 relative.  Weights are fp8
    UNSCALED: sigma(W)=0.02 sits in e4m3's subnormal range whose absolute
    step 2^-10 gives ~2.8% rms relative error - same as a prescaled cast -
    so no dequant multiplies are needed anywhere.
  - Q is pre-scaled by 1/sqrt(HD) via the rope tables, so exp needs no
    scale and the causal mask needs no add: masking is done by multiplying
    probs by 0/1 triangle tiles on the otherwise idle GPSIMD engine.

matmul(out, lhsT, rhs) = lhsT.T @ rhs, contraction over the partition dim.
DoubleRow operands are [K, 2, free] - two k-slabs contracted per
instruction:  scores pair HD halves (64x2), AV/sums pair adjacent sk
tiles (128x2), proj pairs adjacent d tiles (128x2), oproj pairs heads.
"""

import os
from contextlib import ExitStack

import numpy as np
import ml_dtypes

import concourse.bacc as bacc
import concourse.mybir as mybir
import concourse.tile as tile

# ---------------- problem constants (hardcoded per contract) ----------------
B, S, D = 2, 2048, 2048
H, KV, HD = 16, 4, 128
REP = H // KV            # 4 q heads per kv head
NG = KV                  # 4 tensor-parallel groups
NCORES = 8
THETA = 10000.0
SCALE = 1.0 / float(np.sqrt(HD))

P = 128                  # partition dim
SC = 512                 # moving free-dim chunk
NDT = S // P             # 16 tiles of 128 along S or D
NPAIR = NDT // 2         # 8 d-tile pairs
NCH = S // SC            # 4 chunks of 512 along S
NH = REP                 # 4 q-heads per core

FP32 = mybir.dt.float32
BF16 = mybir.dt.bfloat16
F8E4 = mybir.dt.float8e4
DR = mybir.MatmulPerfMode.DoubleRow

NP_BF16 = ml_dtypes.bfloat16
NP_F8E4 = ml_dtypes.float8_e4m3

_CACHE = {}


def _build_program(repeat=1):
    nc = bacc.Bacc("TRN2", target_bir_lowering=False, debug=False)

    # chunk-0 (bf16) inputs: full-D x columns 0..511, full weights
    xbf_d = nc.dram_tensor("xbf", [D, SC], BF16, kind="ExternalInput").ap()
    wqbf_d = nc.dram_tensor("wqbf", [D, NH * HD], BF16, kind="ExternalInput").ap()
    wkbf_d = nc.dram_tensor("wkbf", [D, HD], BF16, kind="ExternalInput").ap()
    wvbf_d = nc.dram_tensor("wvbf", [D, HD], BF16, kind="ExternalInput").ap()
    wobf_d = nc.dram_tensor("wobf", [NH * HD, D], BF16, kind="ExternalInput").ap()
    # fp8 DoubleRow inputs: x cols 512.., d-tile-paired; weights likewise
    x8_d = nc.dram_tensor("x8", [P, NDT * 3 * SC], F8E4, kind="ExternalInput").ap()
    wq8_d = nc.dram_tensor("wq8", [P, NDT * NH * HD], F8E4, kind="ExternalInput").ap()
    wk8_d = nc.dram_tensor("wk8", [P, NDT * HD], F8E4, kind="ExternalInput").ap()
    wv8_d = nc.dram_tensor("wv8", [P, NDT * HD], F8E4, kind="ExternalInput").ap()
    wo8_d = nc.dram_tensor("wo8", [P, 4 * D], F8E4, kind="ExternalInput").ap()
    # rope tables (fp32): q tables pre-scaled by 1/sqrt(HD)
    cosq_d = nc.dram_tensor("cosq", [HD, S], FP32, kind="ExternalInput").ap()
    sinq_d = nc.dram_tensor("sinq", [HD, S], FP32, kind="ExternalInput").ap()
    cosk_d = nc.dram_tensor("cosk", [HD, S], FP32, kind="ExternalInput").ap()
    sink_d = nc.dram_tensor("sink", [HD, S], FP32, kind="ExternalInput").ap()
    # constants: identity (bf16, for transposes), ones, 0/1 triangle fixups
    identb_d = nc.dram_tensor("identb", [P, P], BF16, kind="ExternalInput").ap()
    onesb_d = nc.dram_tensor("onesb", [P, P], BF16, kind="ExternalInput").ap()
    ones8_d = nc.dram_tensor("ones8", [P, 2 * P], F8E4, kind="ExternalInput").ap()
    tri8_d = nc.dram_tensor("tri8", [P, 3 * P], F8E4, kind="ExternalInput").ap()
    trib_d = nc.dram_tensor("trib", [P, 3 * P], BF16, kind="ExternalInput").ap()
    y_d = nc.dram_tensor("y", [S, D], FP32, kind="ExternalOutput").ap()

    with tile.TileContext(nc) as tc, ExitStack() as ctx:
        per = ctx.enter_context(tc.tile_pool(name="per", bufs=1))

        # resident activations
        qt_bf = [per.tile([P, SC], BF16, tag=f"qtb{h}", name=f"qtb{h}")
                 for h in range(NH)]
        qt8 = [per.tile([P, 2 * 3 * SC], F8E4, tag=f"qt8{h}", name=f"qt8{h}")
               for h in range(NH)]
        kt_bf = per.tile([P, SC], BF16, tag="ktb", name="ktb")
        kt8 = per.tile([P, 2 * S], F8E4, tag="kt8", name="kt8")
        v_bf = [per.tile([P, P], BF16, tag=f"vb{k}", name=f"vb{k}")
                for k in range(4)]
        v8 = per.tile([P, NDT * P], F8E4, tag="v8", name="v8")

        # tables + consts
        cosq = per.tile([HD, S], FP32, tag="cosq")
        sinq = per.tile([HD, S], FP32, tag="sinq")
        cosk = per.tile([HD, S], FP32, tag="cosk")
        sink = per.tile([HD, S], FP32, tag="sink")
        identb = per.tile([P, P], BF16, tag="identb")
        onesb = per.tile([P, P], BF16, tag="onesb")
        ones8 = per.tile([P, 2 * P], F8E4, tag="ones8")
        tri8 = per.tile([P, 3 * P], F8E4, tag="tri8")
        trib = per.tile([P, 3 * P], BF16, tag="trib")
        # triangle views: T128 = cols 0:128 (keep b>=a), T256 = cols 128:384
        # (keep b>=128+a)
        t128_8, t256_8 = tri8[:, 0:P], tri8[:, P:3 * P]
        t128_b, t256_b = trib[:, 0:P], trib[:, P:3 * P]

        def load_consts():
            nc.gpsimd.dma_start(identb[:], identb_d[:])
            nc.gpsimd.dma_start(onesb[:], onesb_d[:])
            nc.gpsimd.dma_start(ones8[:], ones8_d[:])
            nc.gpsimd.dma_start(tri8[:], tri8_d[:])
            nc.gpsimd.dma_start(trib[:], trib_d[:])
            nc.gpsimd.dma_start(cosk[:], cosk_d[:])
            nc.gpsimd.dma_start(sink[:], sink_d[:])
            nc.gpsimd.dma_start(cosq[:], cosq_d[:])
            nc.gpsimd.dma_start(sinq[:], sinq_d[:])

        for rep in range(repeat):
            # ============== phase 1: QKV projection + RoPE ==============
            with tc.tile_pool(name="p1", bufs=1) as p1, \
                 tc.tile_pool(name="xin", bufs=2) as xin, \
                 tc.tile_pool(name="rtmp", bufs=3) as rtmp, \
                 tc.tile_pool(name="ps1", bufs=1, space="PSUM") as ps1:

                # bf16 weight slabs (full D, for chunk 0)
                wqs = p1.tile([P, NDT * NH * HD], BF16, tag="wqs")
                wks = p1.tile([P, NDT * HD], BF16, tag="wks")
                wvs = p1.tile([P, NDT * HD], BF16, tag="wvs")
                # fp8 weight slabs (d-tile pairs)
                wq8 = p1.tile([P, NDT * NH * HD], F8E4, tag="wq8")
                wk8 = p1.tile([P, NDT * HD], F8E4, tag="wk8")
                wv8 = p1.tile([P, NDT * HD], F8E4, tag="wv8")

                # chunk-0 x: 4 quarter slabs on the sync queue
                xbf = []
                for qq in range(4):
                    xs = p1.tile([P, 4 * SC], BF16, tag=f"xbf{qq}")
                    nc.sync.dma_start(
                        xs[:].rearrange("p (n s) -> p n s", n=4),
                        xbf_d[qq * 4 * P:(qq + 1) * 4 * P, :]
                        .rearrange("(n p) s -> p n s", p=P))
                    xbf.append(xs)
                nc.scalar.dma_start(
                    wqs[:].rearrange("p (n m) -> p n m", n=NDT),
                    wqbf_d.rearrange("(n p) m -> p n m", p=P))
                nc.scalar.dma_start(
                    wks[:].rearrange("p (n m) -> p n m", n=NDT),
                    wkbf_d.rearrange("(n p) m -> p n m", p=P))
                nc.scalar.dma_start(
                    wvs[:].rearrange("p (n m) -> p n m", n=NDT),
                    wvbf_d.rearrange("(n p) m -> p n m", p=P))
                nc.gpsimd.dma_start(wq8[:], wq8_d[:])
                nc.gpsimd.dma_start(wk8[:], wk8_d[:])
                nc.gpsimd.dma_start(wv8[:], wv8_d[:])
                if rep == 0:
                    load_consts()

                def rope_bf(psum, dst, cosT, sinT, s0):
                    """dst(bf16)[128,SC] = psum*cos + shifthalf(psum)*sin."""
                    t0 = rtmp.tile([P, SC], FP32, tag="t0")
                    t1 = rtmp.tile([P, SC], FP32, tag="t1")
                    nc.vector.tensor_mul(t0[:], psum[:], cosT[:, s0:s0 + SC])
                    nc.vector.tensor_mul(
                        t1[0:64, :], psum[64:128, :], sinT[0:64, s0:s0 + SC])
                    nc.vector.tensor_mul(
                        t1[64:128, :], psum[0:64, :], sinT[64:128, s0:s0 + SC])
                    nc.vector.tensor_add(dst, t0[:], t1[:])

                def rope_f8(psum, dst8, iw, c0, cosT, sinT, s0):
                    """dst8[0:64, i, c0:c0+SC] (fp8 split-half layout) =
                    rope(psum); iw = dst8 free width per i."""
                    t0 = rtmp.tile([P, SC], FP32, tag="t0")
                    t1 = rtmp.tile([P, SC], FP32, tag="t1")
                    nc.vector.tensor_mul(t0[:], psum[:], cosT[:, s0:s0 + SC])
                    nc.vector.tensor_mul(
                        t1[0:64, :], psum[64:128, :], sinT[0:64, s0:s0 + SC])
                    nc.vector.tensor_mul(
                        t1[64:128, :], psum[0:64, :], sinT[64:128, s0:s0 + SC])
                    nc.vector.tensor_add(
                        dst8[0:64, c0:c0 + SC], t0[0:64, :], t1[0:64, :])
                    nc.vector.tensor_add(
                        dst8[0:64, iw + c0:iw + c0 + SC],
                        t0[64:128, :], t1[64:128, :])

                for sc in range(NCH):
                    s0 = sc * SC
                    vT = p1.tile([HD, SC], BF16, tag="vT", bufs=2)
                    if sc == 0:
                        # ---- bf16 chunk ----
                        for m in range(NH + 2):
                            psum = ps1.tile([P, SC], FP32, tag="proj", bufs=4)
                            for k in range(NDT):
                                if m < NH:
                                    lhsT = wqs[:, k * NH * HD + m * HD:
                                               k * NH * HD + (m + 1) * HD]
                                elif m == NH:
                                    lhsT = wks[:, k * HD:(k + 1) * HD]
                                else:
                                    lhsT = wvs[:, k * HD:(k + 1) * HD]
                                nc.tensor.matmul(
                                    psum[:], lhsT,
                                    xbf[k // 4][:, (k % 4) * SC:(k % 4 + 1) * SC],
                                    start=(k == 0), stop=(k == NDT - 1))
                            if m < NH:
                                rope_bf(psum, qt_bf[m][:], cosq, sinq, s0)
                            elif m == NH:
                                # k: bf16 tiles 0..3 AND fp8 split-half copy,
                                # sharing one set of rope products
                                t0 = rtmp.tile([P, SC], FP32, tag="t0")
                                t1 = rtmp.tile([P, SC], FP32, tag="t1")
                                nc.vector.tensor_mul(
                                    t0[:], psum[:], cosk[:, s0:s0 + SC])
                                nc.vector.tensor_mul(
                                    t1[0:64, :], psum[64:128, :],
                                    sink[0:64, s0:s0 + SC])
                                nc.vector.tensor_mul(
                                    t1[64:128, :], psum[0:64, :],
                                    sink[64:128, s0:s0 + SC])
                                nc.vector.tensor_add(kt_bf[:], t0[:], t1[:])
                                nc.vector.tensor_add(
                                    kt8[0:64, s0:s0 + SC],
                                    t0[0:64, :], t1[0:64, :])
                                nc.vector.tensor_add(
                                    kt8[0:64, S + s0:S + s0 + SC],
                                    t0[64:128, :], t1[64:128, :])
                            else:
                                nc.vector.tensor_copy(vT[:], psum[:])
                    else:
                        # ---- fp8 DoubleRow chunk ----
                        x8s = xin.tile([P, NDT * SC], F8E4, tag="x8s")
                        nc.sync.dma_start(
                            x8s[:].rearrange("p (n s) -> p n s", n=NDT),
                            x8_d.rearrange("p (n s) -> p n s", n=NDT)
                            [:, :, (sc - 1) * SC:sc * SC])
                        for m in range(NH + 2):
                            psum = ps1.tile([P, SC], FP32, tag="proj", bufs=4)
                            for j in range(NPAIR):
                                if m < NH:
                                    lhsT = (wq8[:, j * 2 * NH * HD:
                                                (j + 1) * 2 * NH * HD]
                                            .rearrange("p (i c) -> p i c", i=2)
                                            [:, :, m * HD:(m + 1) * HD])
                                elif m == NH:
                                    lhsT = (wk8[:, j * 2 * HD:(j + 1) * 2 * HD]
                                            .rearrange("p (i c) -> p i c", i=2))
                                else:
                                    lhsT = (wv8[:, j * 2 * HD:(j + 1) * 2 * HD]
                                            .rearrange("p (i c) -> p i c", i=2))
                                nc.tensor.matmul(
                                    psum[:], lhsT,
                                    x8s[:, j * 2 * SC:(j + 1) * 2 * SC]
                                    .rearrange("p (i s) -> p i s", i=2),
                                    start=(j == 0), stop=(j == NPAIR - 1),
                                    perf_mode=DR)
                            if m < NH:
                                rope_f8(psum, qt8[m], 3 * SC, (sc - 1) * SC,
                                        cosq, sinq, s0)
                            elif m == NH:
                                rope_f8(psum, kt8, S, s0, cosk, sink, s0)
                            else:
                                nc.vector.tensor_copy(vT[:], psum[:])

                    # transpose V^T chunk -> v tiles [sk=128, HD]
                    for kk in range(SC // P):
                        k = sc * (SC // P) + kk
                        ps_t = ps1.tile([P, P], BF16, tag="vt")
                        nc.tensor.transpose(
                            ps_t[:], vT[:, kk * P:(kk + 1) * P], identb[:])
                        if k < 4:
                            nc.vector.tensor_copy(v_bf[k][:], ps_t[:])
                        nc.vector.tensor_copy(v8[:, k * P:(k + 1) * P], ps_t[:])

            # ========== phase 2: attention + output projection ==========
            with tc.tile_pool(name="p2", bufs=1) as p2, \
                 tc.tile_pool(name="ptb", bufs=3) as ptbp, \
                 tc.tile_pool(name="pt8", bufs=10) as pt8p, \
                 tc.tile_pool(name="nrm", bufs=2) as nrm, \
                 tc.tile_pool(name="yst", bufs=3) as yst, \
                 tc.tile_pool(name="ps2", bufs=1, space="PSUM") as ps2, \
                 tc.tile_pool(name="pss", bufs=1, space="PSUM") as pss:

                wos = p2.tile([P, NH * D], BF16, tag="wos")
                nc.scalar.dma_start(
                    wos[:].rearrange("p (n d) -> p n d", n=NH),
                    wobf_d.rearrange("(n p) d -> p n d", p=P))
                wo8 = p2.tile([P, 4 * D], F8E4, tag="wo8")
                nc.gpsimd.dma_start(wo8[:], wo8_d[:])
                outT_bf = [p2.tile([P, SC], BF16, tag=f"otb{h}", name=f"otb{h}")
                           for h in range(NH)]
                outT8 = p2.tile([P, 4 * SC], F8E4, tag="ot8", name="ot8")

                # pre-zero the pt slab pools so stale-gap regions multiplied
                # by the 0/1 fixup tiles are finite on first use
                for _ in range(3):
                    tb = ptbp.tile([P, 2 * SC], BF16, tag="ptb")
                    nc.gpsimd.memset(tb[:], 0.0)
                for _ in range(10):
                    t8 = pt8p.tile([P, 2 * SC], F8E4, tag="pt8")
                    nc.gpsimd.memset(t8[:], 0.0)

                for c in range(NCH):
                    q0 = c * SC
                    nk = 4 * c + 4          # active sk tiles (causal)
                    npair = nk // 2
                    for h in range(NH):
                        sums_ps = pss.tile([P, SC], FP32, tag="sums", bufs=1)
                        av_ps = pss.tile([P, SC], FP32, tag="av", bufs=2)
                        slabs = []      # (slab, pair_off)
                        for kp in range(npair):
                            if c == 0:
                                slab = ptbp.tile([P, 2 * SC], BF16, tag="ptb")
                            else:
                                slab = pt8p.tile([P, 2 * SC], F8E4, tag="pt8")
                            pair_off = max(0, (2 * kp - 4 * c) * P)
                            for i in range(2):
                                k = 2 * kp + i
                                # true off: masked-out left columns skipped
                                off = max(0, (k - 4 * c) * P)
                                sc_ps = ps2.tile([P, SC], FP32, tag="sc",
                                                 bufs=3)
                                if c == 0:
                                    nc.tensor.matmul(
                                        sc_ps[:, off:],
                                        kt_bf[:, k * P:(k + 1) * P],
                                        qt_bf[h][:, q0 + off:q0 + SC],
                                        start=True, stop=True)
                                else:
                                    lhsT = (kt8[0:64, :]
                                            .rearrange("p (i s) -> p i s", i=2)
                                            [:, :, k * P:(k + 1) * P])
                                    rhs = (qt8[h][0:64, :]
                                           .rearrange("p (i s) -> p i s", i=2)
                                           [:, :, q0 - SC + off:q0 - SC + SC])
                                    nc.tensor.matmul(
                                        sc_ps[:, off:], lhsT, rhs,
                                        start=True, stop=True, perf_mode=DR)
                                nc.scalar.activation(
                                    slab[:, i * SC + off:(i + 1) * SC],
                                    sc_ps[:, off:],
                                    mybir.ActivationFunctionType.Exp)
                                if k >= 4 * c:
                                    # causal fixup: multiply by 0/1 triangle
                                    j = k - 4 * c
                                    tri = (t128_b if c == 0 else t128_8) \
                                        if i == 0 else \
                                        (t256_b if c == 0 else t256_8)
                                    w = P if i == 0 else 2 * P
                                    f0 = i * SC + pair_off
                                    nc.gpsimd.tensor_mul(
                                        slab[:, f0:f0 + w],
                                        slab[:, f0:f0 + w],
                                        tri[:, 0:w])
                            slabs.append((slab, pair_off))

                        # row sums + AV, accumulated over pairs
                        for kp, (slab, off) in enumerate(slabs):
                            st = (kp == 0)
                            sp = (kp == npair - 1)
                            if c == 0:
                                for i in range(2):
                                    k = 2 * kp + i
                                    nc.tensor.matmul(
                                        sums_ps[:, off:], onesb[:],
                                        slab[:, i * SC + off:(i + 1) * SC],
                                        start=(st and i == 0),
                                        stop=(sp and i == 1))
                                for i in range(2):
                                    k = 2 * kp + i
                                    nc.tensor.matmul(
                                        av_ps[:, off:], v_bf[k][:],
                                        slab[:, i * SC + off:(i + 1) * SC],
                                        start=(st and i == 0),
                                        stop=(sp and i == 1))
                            else:
                                rhs = (slab[:]
                                       .rearrange("p (i s) -> p i s", i=2)
                                       [:, :, off:])
                                nc.tensor.matmul(
                                    sums_ps[:, off:],
                                    ones8[:].rearrange(
                                        "p (i c) -> p i c", i=2),
                                    rhs, start=st, stop=sp, perf_mode=DR)
                                nc.tensor.matmul(
                                    av_ps[:, off:],
                                    v8[:, kp * 2 * P:(kp + 1) * 2 * P]
                                    .rearrange("p (i c) -> p i c", i=2),
                                    rhs, start=st, stop=sp, perf_mode=DR)

                        recip = nrm.tile([P, SC], FP32, tag="recip")
                        nc.vector.reciprocal(recip[:], sums_ps[:])
                        if c == 0:
                            nc.vector.tensor_mul(
                                outT_bf[h][:], av_ps[:], recip[:])
                        else:
                            nc.vector.tensor_mul(
                                outT8[:, h * SC:(h + 1) * SC],
                                av_ps[:], recip[:])

                    # output projection for this sq chunk
                    for t in range(SC // P):
                        yslab = yst.tile([P, D], FP32, tag="yslab")
                        for dci in range(NCH):
                            d0 = dci * SC
                            y_ps = ps2.tile([P, SC], FP32, tag="y", bufs=2)
                            if c == 0:
                                for hh in range(NH):
                                    nc.tensor.matmul(
                                        y_ps[:],
                                        outT_bf[hh][:, t * P:(t + 1) * P],
                                        wos[:, hh * D + d0:hh * D + d0 + SC],
                                        start=(hh == 0), stop=(hh == NH - 1))
                            else:
                                for hp in range(2):
                                    lhsT = (outT8[:, hp * 2 * SC:
                                                  (hp + 1) * 2 * SC]
                                            .rearrange("p (i s) -> p i s", i=2)
                                            [:, :, t * P:(t + 1) * P])
                                    rhs = (wo8[:, hp * 2 * D:(hp + 1) * 2 * D]
                                           .rearrange("p (i d) -> p i d", i=2)
                                           [:, :, d0:d0 + SC])
                                    nc.tensor.matmul(
                                        y_ps[:], lhsT, rhs,
                                        start=(hp == 0), stop=(hp == 1),
                                        perf_mode=DR)
                            nc.vector.tensor_copy(yslab[:, d0:d0 + SC], y_ps[:])
                        row0 = q0 + t * P
                        nc.sync.dma_start(y_d[row0:row0 + P, :], yslab[:])

    nc.compile()
    return nc


def _host_tables():
    inv_freq = 1.0 / (THETA ** (np.arange(0, HD, 2, dtype=np.float32) / HD))
    t = np.arange(S, dtype=np.float32)
    freqs = t[:, None] * inv_freq[None, :]              # [S, HD/2]
    emb = np.concatenate([freqs, freqs], axis=-1)       # [S, HD]
    cos = np.cos(emb).astype(np.float32)
    sin = np.sin(emb).astype(np.float32)
    cosT = np.ascontiguousarray(cos.T)                  # [HD, S]
    sinT = np.ascontiguousarray(sin.T)
    sinrotT = sinT.copy()
    sinrotT[0:HD // 2] = -sinT[0:HD // 2]
    return cosT, sinrotT


def get_program(repeat=1):
    key = ("nc", repeat)
    if key not in _CACHE:
        _CACHE[key] = _build_program(repeat)
    return _CACHE[key]


def _pack_pairs(w):
    """[D, M] -> [128, NPAIR*2*M] fp8, d-tile pairs interleaved for DoubleRow."""
    Dn, M = w.shape
    r = w.reshape(NPAIR, 2, P, M).transpose(2, 0, 1, 3).reshape(P, NPAIR * 2 * M)
    return np.ascontiguousarray(r).astype(NP_F8E4)


def make_in_maps(x, wq, wk, wv, wo, mask):
    x = np.asarray(x, dtype=np.float32)
    wq = np.asarray(wq, dtype=np.float32)
    wk = np.asarray(wk, dtype=np.float32)
    wv = np.asarray(wv, dtype=np.float32)
    wo = np.asarray(wo, dtype=np.float32)

    cosT, sinrotT = _host_tables()
    cosq = np.ascontiguousarray(cosT * SCALE)
    sinq = np.ascontiguousarray(sinrotT * SCALE)

    identb = np.eye(P, dtype=NP_BF16)
    onesb = np.ones((P, P), dtype=NP_BF16)
    ones8 = np.ones((P, 2 * P), dtype=NP_F8E4)
    a = np.arange(P)[:, None]
    t128 = (np.arange(P)[None, :] >= a).astype(np.float32)
    t256 = (np.arange(2 * P)[None, :] >= P + a).astype(np.float32)
    tri = np.concatenate([t128, t256], axis=1)
    tri8 = tri.astype(NP_F8E4)
    trib = tri.astype(NP_BF16)

    # per-batch x packs
    xbf, x8 = [], []
    for b in range(B):
        xT = np.ascontiguousarray(x[b].T)               # [D, S]
        xbf.append(np.ascontiguousarray(xT[:, :SC]).astype(NP_BF16))
        xr = (xT[:, SC:].reshape(NDT, P, 3 * SC)
              .transpose(1, 0, 2).reshape(P, NDT * 3 * SC))
        x8.append(np.ascontiguousarray(xr).astype(NP_F8E4))

    # per-group weight packs
    packs = []
    for g in range(NG):
        qc0, kc0 = g * NH * HD, g * HD
        wqg = wq[:, qc0:qc0 + NH * HD]
        wkg = wk[:, kc0:kc0 + HD]
        wvg = wv[:, kc0:kc0 + HD]
        wog = wo[qc0:qc0 + NH * HD, :]
        # wo8: head-pairs: [p, hp, i, d]
        wo8 = (wog.reshape(2, 2, P, D).transpose(2, 0, 1, 3)
               .reshape(P, 4 * D))
        packs.append({
            "wqbf": np.ascontiguousarray(wqg).astype(NP_BF16),
            "wkbf": np.ascontiguousarray(wkg).astype(NP_BF16),
            "wvbf": np.ascontiguousarray(wvg).astype(NP_BF16),
            "wobf": np.ascontiguousarray(wog).astype(NP_BF16),
            "wq8": _pack_pairs(wqg),
            "wk8": _pack_pairs(wkg),
            "wv8": _pack_pairs(wvg),
            "wo8": np.ascontiguousarray(wo8).astype(NP_F8E4),
        })

    in_maps = []
    for c in range(NCORES):
        b, g = c // NG, c % NG
        m = {
            "xbf": xbf[b], "x8": x8[b],
            "cosq": cosq, "sinq": sinq, "cosk": cosT, "sink": sinrotT,
            "identb": identb, "onesb": onesb, "ones8": ones8,
            "tri8": tri8, "trib": trib,
        }
        m.update(packs[g])
        in_maps.append(m)
    return in_maps


LAST_RESULTS = None


def _make_exec(nc):
    """Mirror run_bass_via_pjrt's multi-core path, but keep the jitted
    executable so repeated (timed) dispatches skip retrace/reload."""
    import jax
    from jax.experimental.shard_map import shard_map
    from jax.sharding import Mesh, PartitionSpec

    from concourse import bass2jax, mybir as _mybir

    bass2jax.install_neuronx_cc_hook()
    partition_name = (
        nc.partition_id_tensor.name if nc.partition_id_tensor else None)
    in_names, out_names, out_avals, zero_outs = [], [], [], []
    for alloc in nc.m.functions[0].allocations:
        if not isinstance(alloc, _mybir.MemoryLocationSet):
            continue
        name = alloc.memorylocations[0].name
        if alloc.kind == "ExternalInput":
            if name != partition_name:
                in_names.append(name)
        elif alloc.kind == "ExternalOutput":
            shape = tuple(alloc.tensor_shape)
            dtype = _mybir.dt.np(alloc.dtype)
            out_names.append(name)
            out_avals.append(jax.core.ShapedArray(shape, dtype))
            zero_outs.append(np.zeros(shape, dtype))
    n_params = len(in_names)
    n_outs = len(out_avals)
    all_in_names = list(in_names) + list(out_names)
    if partition_name is not None:
        all_in_names.append(partition_name)
    donate = tuple(range(n_params, n_params + n_outs))

    def _body(*args):
        operands = list(args)
        if partition_name is not None:
            operands.append(bass2jax.partition_id_tensor())
        outs = bass2jax._bass_exec_p.bind(
            *operands,
            out_avals=tuple(out_avals),
            in_names=tuple(all_in_names),
            out_names=tuple(out_names),
            lowering_input_output_aliases=(),
            sim_require_finite=True,
            sim_require_nnan=True,
            nc=nc,
        )
        return tuple(outs)

    devices = jax.devices()[:NCORES]
    mesh = Mesh(np.asarray(devices), ("core",))
    sharded = jax.jit(
        shard_map(
            _body, mesh=mesh,
            in_specs=(PartitionSpec("core"),) * (n_params + n_outs),
            out_specs=(PartitionSpec("core"),) * n_outs,
            check_rep=False,
        ),
        donate_argnums=donate, keep_unused=True,
    )
    return {
        "fn": sharded, "in_names": in_names, "out_names": out_names,
        "out_avals": out_avals, "zero_outs": zero_outs, "mesh": mesh,
    }


def get_exec(repeat=1):
    key = ("exec", repeat)
    if key not in _CACHE:
        _CACHE[key] = _make_exec(get_program(repeat))
    return _CACHE[key]


def _concat_inputs(ex, in_maps):
    return [
        np.concatenate([np.asarray(in_maps[c][name]) for c in range(NCORES)],
                       axis=0)
        for name in ex["in_names"]
    ]


def _concat_zeros(ex):
    return [
        np.zeros((NCORES * z.shape[0], *z.shape[1:]), z.dtype)
        for z in ex["zero_outs"]
    ]


def run_on_device(in_maps, repeat=1):
    """One dispatch; returns per-core output dicts (numpy)."""
    ex = get_exec(repeat)
    out_arrs = ex["fn"](*_concat_inputs(ex, in_maps), *_concat_zeros(ex))
    res = []
    for c in range(NCORES):
        res.append({
            name: np.asarray(out_arrs[i]).reshape(
                NCORES, *ex["out_avals"][i].shape)[c]
            for i, name in enumerate(ex["out_names"])
        })
    return res


def bench(in_maps, iters=5, repeat=1):
    """Timed repeated dispatch: inputs pre-placed on device, fresh donated
    zero output buffers pre-placed per iteration. Returns list of wall ns."""
    import time

    import jax
    from jax.sharding import NamedSharding, PartitionSpec

    ex = get_exec(repeat)
    sh = NamedSharding(ex["mesh"], PartitionSpec("core"))
    dev_in = [jax.device_put(a, sh) for a in _concat_inputs(ex, in_maps)]
    zsets = [[jax.device_put(z, sh) for z in _concat_zeros(ex)]
             for _ in range(iters + 1)]
    jax.block_until_ready(dev_in)
    jax.block_until_ready(zsets)
    out = ex["fn"](*dev_in, *zsets[0])       # warm-up
    jax.block_until_ready(out)
    times = []
    for i in range(iters):
        t0 = time.perf_counter()
        out = ex["fn"](*dev_in, *zsets[i + 1])
        jax.block_until_ready(out)
        times.append((time.perf_counter() - t0) * 1e9)
    return times


def bench_slope(in_maps, iters=8, r_hi=4):
    """Per-iteration kernel time via slope: (T(r_hi) - T(1)) / (r_hi - 1).
    Immune to constant dispatch overhead."""
    t1 = bench(in_maps, iters=iters, repeat=1)
    th = bench(in_maps, iters=iters, repeat=r_hi)
    t1m, thm = np.median(t1), np.median(th)
    t1b, thb = min(t1), min(th)
    return {
        "t1": t1, "th": th,
        "exec_ns_median": (thm - t1m) / (r_hi - 1),
        "exec_ns_min": (thb - t1b) / (r_hi - 1),
    }


def kernel(x, wq, wk, wv, wo, mask):
    """Full inputs in, full output out; shards over the 8 NeuronCores."""
    global LAST_RESULTS
    from concourse import bass_utils

    nc = get_program()
    in_maps = make_in_maps(x, wq, wk, wv, wo, mask)
    res = bass_utils.run_bass_kernel_spmd(
        nc, in_maps, core_ids=list(range(NCORES)))
    LAST_RESULTS = res
    out = np.zeros((B, S, D), dtype=np.float32)
    for c in range(NCORES):
        b = c // NG
        out[b] += res.results[c]["y"]
    return out


# revision 10
# speedup vs baseline: 1.3412x; 1.3412x over previous
"""Trainium2 Bass kernel for GQA attention (B=2,S=2048,D=2048,H=16,KV=4,HD=128)
with RoPE + causal mask, sharded over 8 NeuronCores:
  2-way data parallel over batch x 4-way tensor parallel over KV groups.

Mixed precision, engine-balanced against HW-measured costs (Act exp
~2ns/col, 64-partition DoubleRow matmuls slow, gpsimd ucode ~5us/launch):
  - rows 0..511 (chunk 0): bf16 path with accurate Act exp.
  - rows 512..2047: proj / AV / row-sums / out-proj in fp8e4 DoubleRow
    (128-partition pairing only); scores in bf16.
  - probs for rows 512+ are produced at HALF scale (cancelled by softmax
    normalization) either by Act exp (bias=-ln2, fp8 out) or by a DVE
    scalar_tensor_tensor "Schraudolph" exp: u8 = sat(round(x*8/ln2 + 48)),
    whose bytes ARE the e4m3 bit pattern of exp(x)/2.  The causal mask is
    folded into the stt bias tile (-1000 -> saturates to 0 probability).
  - Q is pre-scaled by 1/sqrt(HD) via its rope tables; fp8 weights cast
    UNSCALED (sigma(W)=0.02 lands in e4m3's subnormal range; ~2.8% rms,
    same as a prescaled cast) so no dequant multiplies exist anywhere.

matmul(out, lhsT, rhs) = lhsT.T @ rhs, contraction over the partition dim.
DoubleRow operands are [K, 2, free] - two k-slabs contracted per
instruction:  scores pair HD halves (64x2), AV/sums pair adjacent sk
tiles (128x2), proj pairs adjacent d tiles (128x2), oproj pairs heads.
"""

import os
from contextlib import ExitStack

import numpy as np
import ml_dtypes

import concourse.bacc as bacc
import concourse.mybir as mybir
import concourse.tile as tile

# ---------------- problem constants (hardcoded per contract) ----------------
B, S, D = 2, 2048, 2048
H, KV, HD = 16, 4, 128
REP = H // KV            # 4 q heads per kv head
NG = KV                  # 4 tensor-parallel groups
NCORES = 8
THETA = 10000.0
SCALE = 1.0 / float(np.sqrt(HD))

P = 128                  # partition dim
SC = 512                 # moving free-dim chunk
NDT = S // P             # 16 tiles of 128 along S or D
NPAIR = NDT // 2         # 8 d-tile pairs
NCH = S // SC            # 4 chunks of 512 along S
NH = REP                 # 4 q-heads per core

FP32 = mybir.dt.float32
BF16 = mybir.dt.bfloat16
F8E4 = mybir.dt.float8e4
U8 = mybir.dt.uint8
DR = mybir.MatmulPerfMode.DoubleRow

NP_BF16 = ml_dtypes.bfloat16
NP_F8E4 = ml_dtypes.float8_e4m3

# Schraudolph-in-e4m3: bits = EA*x + EB  ->  value ~= exp(x)/2
EA = 8.0 / float(np.log(2.0))
EB = 48.0
LN2 = float(np.log(2.0))

# every ACT_NTH'th full block runs its exp on Act (rest: DVE stt)
ACT_NTH = int(os.environ.get("KERNEL_ACT_NTH", "3"))

_CACHE = {}


def _build_program(repeat=1):
    nc = bacc.Bacc("TRN2", target_bir_lowering=False, debug=False)

    # chunk-0 (bf16) inputs: full-D x columns 0..511, full weights
    xbf_d = nc.dram_tensor("xbf", [D, SC], BF16, kind="ExternalInput").ap()
    wqbf_d = nc.dram_tensor("wqbf", [D, NH * HD], BF16, kind="ExternalInput").ap()
    wkbf_d = nc.dram_tensor("wkbf", [D, HD], BF16, kind="ExternalInput").ap()
    wvbf_d = nc.dram_tensor("wvbf", [D, HD], BF16, kind="ExternalInput").ap()
    wobf_d = nc.dram_tensor("wobf", [NH * HD, D], BF16, kind="ExternalInput").ap()
    # fp8 DoubleRow inputs: x cols 512.., d-tile-paired; weights likewise
    x8_d = nc.dram_tensor("x8", [P, NDT * 3 * SC], F8E4, kind="ExternalInput").ap()
    wq8_d = nc.dram_tensor("wq8", [P, NDT * NH * HD], F8E4, kind="ExternalInput").ap()
    wk8_d = nc.dram_tensor("wk8", [P, NDT * HD], F8E4, kind="ExternalInput").ap()
    wv8_d = nc.dram_tensor("wv8", [P, NDT * HD], F8E4, kind="ExternalInput").ap()
    wo8_d = nc.dram_tensor("wo8", [P, 4 * D], F8E4, kind="ExternalInput").ap()
    # rope tables (bf16): q tables pre-scaled by 1/sqrt(HD)
    cosq_d = nc.dram_tensor("cosq", [HD, S], BF16, kind="ExternalInput").ap()
    sinq_d = nc.dram_tensor("sinq", [HD, S], BF16, kind="ExternalInput").ap()
    cosk_d = nc.dram_tensor("cosk", [HD, S], BF16, kind="ExternalInput").ap()
    sink_d = nc.dram_tensor("sink", [HD, S], BF16, kind="ExternalInput").ap()
    # constants: identity (bf16, for transposes), ones, 0/1 triangle fixups
    identb_d = nc.dram_tensor("identb", [P, P], BF16, kind="ExternalInput").ap()
    onesb_d = nc.dram_tensor("onesb", [P, P], BF16, kind="ExternalInput").ap()
    ones8_d = nc.dram_tensor("ones8", [P, 2 * P], F8E4, kind="ExternalInput").ap()
    trib_d = nc.dram_tensor("trib", [P, 3 * P], BF16, kind="ExternalInput").ap()
    # stt bias slabs: [bias48(SC) | j0(SC) | j1(SC) | j2(2P) | j3(2P)]
    bias_d = nc.dram_tensor("bias", [P, 3 * SC + 4 * P], BF16,
                            kind="ExternalInput").ap()
    negln2_d = nc.dram_tensor("negln2", [P, 1], FP32, kind="ExternalInput").ap()
    y_d = nc.dram_tensor("y", [S, D], FP32, kind="ExternalOutput").ap()

    with tile.TileContext(nc) as tc, ExitStack() as ctx:
        per = ctx.enter_context(tc.tile_pool(name="per", bufs=1))

        # resident activations: full-length bf16 Q^T (pre-scaled) / K^T
        qt_bf = [per.tile([P, S], BF16, tag=f"qtb{h}", name=f"qtb{h}")
                 for h in range(NH)]
        kt_bf = per.tile([P, S], BF16, tag="ktb", name="ktb")
        v_bf = [per.tile([P, P], BF16, tag=f"vb{k}", name=f"vb{k}")
                for k in range(4)]
        v8 = per.tile([P, NDT * P], F8E4, tag="v8", name="v8")

        # tables + consts
        cosq = per.tile([HD, S], BF16, tag="cosq")
        sinq = per.tile([HD, S], BF16, tag="sinq")
        cosk = per.tile([HD, S], BF16, tag="cosk")
        sink = per.tile([HD, S], BF16, tag="sink")
        identb = per.tile([P, P], BF16, tag="identb")
        onesb = per.tile([P, P], BF16, tag="onesb")
        ones8 = per.tile([P, 2 * P], F8E4, tag="ones8")
        trib = per.tile([P, 3 * P], BF16, tag="trib")
        biasb = per.tile([P, 3 * SC + 4 * P], BF16, tag="biasb")
        negln2 = per.tile([P, 1], FP32, tag="negln2")
        t128_b, t256_b = trib[:, 0:P], trib[:, P:3 * P]
        bias48 = biasb[:, 0:SC]
        # per-diag-j stt bias views (mask folded in)
        bj = [biasb[:, SC:2 * SC], biasb[:, 2 * SC:3 * SC],
              biasb[:, 3 * SC:3 * SC + 2 * P],
              biasb[:, 3 * SC + 2 * P:3 * SC + 4 * P]]

        def load_consts():
            nc.sync.dma_start(identb[:], identb_d[:])
            nc.sync.dma_start(onesb[:], onesb_d[:])
            nc.sync.dma_start(ones8[:], ones8_d[:])
            nc.scalar.dma_start(trib[:], trib_d[:])
            nc.scalar.dma_start(biasb[:], bias_d[:])
            nc.scalar.dma_start(negln2[:], negln2_d[:])
            nc.sync.dma_start(cosq[:], cosq_d[:])
            nc.sync.dma_start(sinq[:], sinq_d[:])
            nc.scalar.dma_start(cosk[:], cosk_d[:])
            nc.scalar.dma_start(sink[:], sink_d[:])

        for rep in range(repeat):
            # ============== phase 1: QKV projection + RoPE ==============
            with tc.tile_pool(name="p1", bufs=1) as p1, \
                 tc.tile_pool(name="xin", bufs=2) as xin, \
                 tc.tile_pool(name="rtmp", bufs=3) as rtmp, \
                 tc.tile_pool(name="ps1", bufs=1, space="PSUM") as ps1:

                # bf16 weight slabs (full D, for chunk 0)
                wqs = p1.tile([P, NDT * NH * HD], BF16, tag="wqs")
                wks = p1.tile([P, NDT * HD], BF16, tag="wks")
                wvs = p1.tile([P, NDT * HD], BF16, tag="wvs")
                # fp8 weight slabs (d-tile pairs)
                wq8 = p1.tile([P, NDT * NH * HD], F8E4, tag="wq8")
                wk8 = p1.tile([P, NDT * HD], F8E4, tag="wk8")
                wv8 = p1.tile([P, NDT * HD], F8E4, tag="wv8")

                # chunk-0 x: 4 quarter slabs on the sync queue
                xbf = []
                for qq in range(4):
                    xs = p1.tile([P, 4 * SC], BF16, tag=f"xbf{qq}")
                    nc.sync.dma_start(
                        xs[:].rearrange("p (n s) -> p n s", n=4),
                        xbf_d[qq * 4 * P:(qq + 1) * 4 * P, :]
                        .rearrange("(n p) s -> p n s", p=P))
                    xbf.append(xs)
                nc.scalar.dma_start(
                    wqs[:].rearrange("p (n m) -> p n m", n=NDT),
                    wqbf_d.rearrange("(n p) m -> p n m", p=P))
                nc.scalar.dma_start(
                    wks[:].rearrange("p (n m) -> p n m", n=NDT),
                    wkbf_d.rearrange("(n p) m -> p n m", p=P))
                nc.scalar.dma_start(
                    wvs[:].rearrange("p (n m) -> p n m", n=NDT),
                    wvbf_d.rearrange("(n p) m -> p n m", p=P))
                nc.gpsimd.dma_start(wq8[:], wq8_d[:])
                nc.gpsimd.dma_start(wk8[:], wk8_d[:])
                nc.gpsimd.dma_start(wv8[:], wv8_d[:])
                if rep == 0:
                    load_consts()

                def rope_bf(psum, dst, cosT, sinT, s0):
                    """dst(bf16) = psum*cos + shifthalf(psum)*sin.
                    PSUM-sourced muls (cross-partition reads require the
                    PSUM operand), bf16 temps, one fast SBUF add."""
                    t0 = rtmp.tile([P, SC], BF16, tag="t0")
                    t1 = rtmp.tile([P, SC], BF16, tag="t1")
                    nc.vector.tensor_mul(t0[:], psum[:], cosT[:, s0:s0 + SC])
                    nc.vector.tensor_mul(
                        t1[0:64, :], psum[64:128, :], sinT[0:64, s0:s0 + SC])
                    nc.vector.tensor_mul(
                        t1[64:128, :], psum[0:64, :], sinT[64:128, s0:s0 + SC])
                    nc.vector.tensor_add(dst, t0[:], t1[:])

                for sc in range(NCH):
                    s0 = sc * SC
                    vT = p1.tile([HD, SC], BF16, tag="vT", bufs=2)
                    if sc == 0:
                        # ---- bf16 chunk ----
                        for m in range(NH + 2):
                            psum = ps1.tile([P, SC], FP32, tag="proj", bufs=4)
                            for k in range(NDT):
                                if m < NH:
                                    lhsT = wqs[:, k * NH * HD + m * HD:
                                               k * NH * HD + (m + 1) * HD]
                                elif m == NH:
                                    lhsT = wks[:, k * HD:(k + 1) * HD]
                                else:
                                    lhsT = wvs[:, k * HD:(k + 1) * HD]
                                nc.tensor.matmul(
                                    psum[:], lhsT,
                                    xbf[k // 4][:, (k % 4) * SC:(k % 4 + 1) * SC],
                                    start=(k == 0), stop=(k == NDT - 1))
                            if m < NH:
                                rope_bf(psum, qt_bf[m][:, s0:s0 + SC],
                                        cosq, sinq, s0)
                            elif m == NH:
                                rope_bf(psum, kt_bf[:, s0:s0 + SC],
                                        cosk, sink, s0)
                            else:
                                nc.vector.tensor_copy(vT[:], psum[:])
                    else:
                        # ---- fp8 DoubleRow chunk ----
                        x8s = xin.tile([P, NDT * SC], F8E4, tag="x8s")
                        nc.sync.dma_start(
                            x8s[:].rearrange("p (n s) -> p n s", n=NDT),
                            x8_d.rearrange("p (n s) -> p n s", n=NDT)
                            [:, :, (sc - 1) * SC:sc * SC])
                        for m in range(NH + 2):
                            psum = ps1.tile([P, SC], FP32, tag="proj", bufs=4)
                            for j in range(NPAIR):
                                if m < NH:
                                    lhsT = (wq8[:, j * 2 * NH * HD:
                                                (j + 1) * 2 * NH * HD]
                                            .rearrange("p (i c) -> p i c", i=2)
                                            [:, :, m * HD:(m + 1) * HD])
                                elif m == NH:
                                    lhsT = (wk8[:, j * 2 * HD:(j + 1) * 2 * HD]
                                            .rearrange("p (i c) -> p i c", i=2))
                                else:
                                    lhsT = (wv8[:, j * 2 * HD:(j + 1) * 2 * HD]
                                            .rearrange("p (i c) -> p i c", i=2))
                                nc.tensor.matmul(
                                    psum[:], lhsT,
                                    x8s[:, j * 2 * SC:(j + 1) * 2 * SC]
                                    .rearrange("p (i s) -> p i s", i=2),
                                    start=(j == 0), stop=(j == NPAIR - 1),
                                    perf_mode=DR)
                            if m < NH:
                                rope_bf(psum, qt_bf[m][:, s0:s0 + SC],
                                        cosq, sinq, s0)
                            elif m == NH:
                                rope_bf(psum, kt_bf[:, s0:s0 + SC],
                                        cosk, sink, s0)
                            else:
                                nc.vector.tensor_copy(vT[:], psum[:])

                    # transpose V^T chunk -> v tiles [sk=128, HD]
                    for kk in range(SC // P):
                        k = sc * (SC // P) + kk
                        ps_t = ps1.tile([P, P], BF16, tag="vt")
                        nc.tensor.transpose(
                            ps_t[:], vT[:, kk * P:(kk + 1) * P], identb[:])
                        if k < 4:
                            nc.vector.tensor_copy(v_bf[k][:], ps_t[:])
                        nc.vector.tensor_copy(v8[:, k * P:(k + 1) * P], ps_t[:])

            # ========== phase 2: attention + output projection ==========
            with tc.tile_pool(name="p2", bufs=1) as p2, \
                 tc.tile_pool(name="ptb", bufs=3) as ptbp, \
                 tc.tile_pool(name="pt8", bufs=10) as pt8p, \
                 tc.tile_pool(name="nrm", bufs=2) as nrm, \
                 tc.tile_pool(name="yst", bufs=3) as yst, \
                 tc.tile_pool(name="ps2", bufs=1, space="PSUM") as ps2, \
                 tc.tile_pool(name="pss", bufs=1, space="PSUM") as pss:

                wos = p2.tile([P, NH * D], BF16, tag="wos")
                nc.gpsimd.dma_start(
                    wos[:].rearrange("p (n d) -> p n d", n=NH),
                    wobf_d.rearrange("(n p) d -> p n d", p=P))
                wo8 = p2.tile([P, 4 * D], F8E4, tag="wo8")
                nc.gpsimd.dma_start(wo8[:], wo8_d[:])
                outT_bf = [p2.tile([P, SC], BF16, tag=f"otb{h}", name=f"otb{h}")
                           for h in range(NH)]
                outT8 = p2.tile([P, 4 * SC], F8E4, tag="ot8", name="ot8")

                # chunk-0 pt slabs get 0/1 fixup-multiplies over stale gap
                # regions -> pre-zero that pool once (stt bias handles fp8)
                for _ in range(3):
                    tb = ptbp.tile([P, 2 * SC], BF16, tag="ptb")
                    nc.vector.memset(tb[:], 0.0)

                for c in range(NCH):
                    q0 = c * SC
                    nk = 4 * c + 4          # active sk tiles (causal)
                    npair = nk // 2
                    for h in range(NH):
                        sums_ps = pss.tile([P, SC], FP32, tag="sums", bufs=1)
                        av_ps = pss.tile([P, SC], FP32, tag="av", bufs=2)
                        slabs = []      # (slab, pair_off)
                        for kp in range(npair):
                            if c == 0:
                                slab = ptbp.tile([P, 2 * SC], BF16, tag="ptb")
                            else:
                                slab = pt8p.tile([P, 2 * SC], F8E4, tag="pt8")
                            pair_off = max(0, (2 * kp - 4 * c) * P)
                            for i in range(2):
                                k = 2 * kp + i
                                off = pair_off  # diag mm covers whole region
                                j = k - 4 * c
                                sc_ps = ps2.tile([P, SC], FP32, tag="sc",
                                                 bufs=3)
                                nc.tensor.matmul(
                                    sc_ps[:, off:],
                                    kt_bf[:, k * P:(k + 1) * P],
                                    qt_bf[h][:, q0 + off:q0 + SC],
                                    start=True, stop=True)
                                dst = slab[:, i * SC + off:(i + 1) * SC]
                                if c == 0:
                                    # accurate Act exp + 0/1 triangle fixup
                                    nc.scalar.activation(
                                        dst, sc_ps[:, off:],
                                        mybir.ActivationFunctionType.Exp)
                                    tri = t128_b if i == 0 else t256_b
                                    w = P if i == 0 else 2 * P
                                    f0 = i * SC + pair_off
                                    nc.vector.tensor_mul(
                                        slab[:, f0:f0 + w],
                                        slab[:, f0:f0 + w], tri[:, 0:w])
                                elif j < 0 and k % ACT_NTH == ACT_NTH - 1:
                                    # full block on Act: exp(x)/2, fp8 out
                                    nc.scalar.activation(
                                        dst, sc_ps[:, off:],
                                        mybir.ActivationFunctionType.Exp,
                                        bias=negln2[:])
                                else:
                                    # DVE stt schraudolph (mask in bias)
                                    bias = (bias48 if j < 0
                                            else bj[j][:, 0:SC - off])
                                    nc.vector.scalar_tensor_tensor(
                                        dst.bitcast(U8), sc_ps[:, off:],
                                        EA, bias,
                                        op0=mybir.AluOpType.mult,
                                        op1=mybir.AluOpType.add)
                            slabs.append((slab, pair_off))

                        # row sums + AV, accumulated over pairs
                        for kp, (slab, off) in enumerate(slabs):
                            st = (kp == 0)
                            sp = (kp == npair - 1)
                            if c == 0:
                                for i in range(2):
                                    k = 2 * kp + i
                                    nc.tensor.matmul(
                                        sums_ps[:, off:], onesb[:],
                                        slab[:, i * SC + off:(i + 1) * SC],
                                        start=(st and i == 0),
                                        stop=(sp and i == 1))
                                for i in range(2):
                                    k = 2 * kp + i
                                    nc.tensor.matmul(
                                        av_ps[:, off:], v_bf[k][:],
                                        slab[:, i * SC + off:(i + 1) * SC],
                                        start=(st and i == 0),
                                        stop=(sp and i == 1))
                            else:
                                rhs = (slab[:]
                                       .rearrange("p (i s) -> p i s", i=2)
                                       [:, :, off:])
                                nc.tensor.matmul(
                                    sums_ps[:, off:],
                                    ones8[:].rearrange(
                                        "p (i c) -> p i c", i=2),
                                    rhs, start=st, stop=sp, perf_mode=DR)
                                nc.tensor.matmul(
                                    av_ps[:, off:],
                                    v8[:, kp * 2 * P:(kp + 1) * 2 * P]
                                    .rearrange("p (i c) -> p i c", i=2),
                                    rhs, start=st, stop=sp, perf_mode=DR)

                        recip = nrm.tile([P, SC], FP32, tag="recip")
                        nc.vector.reciprocal(recip[:], sums_ps[:])
                        if c == 0:
                            nc.vector.tensor_mul(
                                outT_bf[h][:], av_ps[:], recip[:])
                        else:
                            nc.vector.tensor_mul(
                                outT8[:, h * SC:(h + 1) * SC],
                                av_ps[:], recip[:])

                    # output projection for this sq chunk
                    for t in range(SC // P):
                        yslab = yst.tile([P, D], FP32, tag="yslab")
                        for dci in range(NCH):
                            d0 = dci * SC
                            y_ps = ps2.tile([P, SC], FP32, tag="y", bufs=2)
                            if c == 0:
                                for hh in range(NH):
                                    nc.tensor.matmul(
                                        y_ps[:],
                                        outT_bf[hh][:, t * P:(t + 1) * P],
                                        wos[:, hh * D + d0:hh * D + d0 + SC],
                                        start=(hh == 0), stop=(hh == NH - 1))
                            else:
                                for hp in range(2):
                                    lhsT = (outT8[:, hp * 2 * SC:
                                                  (hp + 1) * 2 * SC]
                                            .rearrange("p (i s) -> p i s", i=2)
                                            [:, :, t * P:(t + 1) * P])
                                    rhs = (wo8[:, hp * 2 * D:(hp + 1) * 2 * D]
                                           .rearrange("p (i d) -> p i d", i=2)
                                           [:, :, d0:d0 + SC])
                                    nc.tensor.matmul(
                                        y_ps[:], lhsT, rhs,
                                        start=(hp == 0), stop=(hp == 1),
                                        perf_mode=DR)
                            if (t * NCH + dci) % 2 == 0:
                                nc.vector.tensor_copy(
                                    yslab[:, d0:d0 + SC], y_ps[:])
                            else:
                                nc.scalar.activation(
                                    yslab[:, d0:d0 + SC], y_ps[:],
                                    mybir.ActivationFunctionType.Copy)
                        row0 = q0 + t * P
                        nc.sync.dma_start(y_d[row0:row0 + P, :], yslab[:])

    nc.compile()
    return nc


def _host_tables():
    inv_freq = 1.0 / (THETA ** (np.arange(0, HD, 2, dtype=np.float32) / HD))
    t = np.arange(S, dtype=np.float32)
    freqs = t[:, None] * inv_freq[None, :]              # [S, HD/2]
    emb = np.concatenate([freqs, freqs], axis=-1)       # [S, HD]
    cos = np.cos(emb).astype(np.float32)
    sin = np.sin(emb).astype(np.float32)
    cosT = np.ascontiguousarray(cos.T)                  # [HD, S]
    sinT = np.ascontiguousarray(sin.T)
    sinrotT = sinT.copy()
    sinrotT[0:HD // 2] = -sinT[0:HD // 2]
    return cosT, sinrotT


def get_program(repeat=1):
    key = ("nc", repeat, ACT_NTH)
    if key not in _CACHE:
        _CACHE[key] = _build_program(repeat)
    return _CACHE[key]


def _pack_pairs(w):
    """[D, M] -> [128, NPAIR*2*M] fp8, d-tile pairs interleaved for DoubleRow."""
    Dn, M = w.shape
    r = w.reshape(NPAIR, 2, P, M).transpose(2, 0, 1, 3).reshape(P, NPAIR * 2 * M)
    return np.ascontiguousarray(r).astype(NP_F8E4)


def make_in_maps(x, wq, wk, wv, wo, mask):
    x = np.asarray(x, dtype=np.float32)
    wq = np.asarray(wq, dtype=np.float32)
    wk = np.asarray(wk, dtype=np.float32)
    wv = np.asarray(wv, dtype=np.float32)
    wo = np.asarray(wo, dtype=np.float32)

    cosT, sinrotT = _host_tables()
    cosq = np.ascontiguousarray(cosT * SCALE).astype(NP_BF16)
    sinq = np.ascontiguousarray(sinrotT * SCALE).astype(NP_BF16)
    coskb = cosT.astype(NP_BF16)
    sinkb = sinrotT.astype(NP_BF16)

    identb = np.eye(P, dtype=NP_BF16)
    onesb = np.ones((P, P), dtype=NP_BF16)
    ones8 = np.ones((P, 2 * P), dtype=NP_F8E4)
    a = np.arange(P)[:, None]
    t128 = (np.arange(P)[None, :] >= a).astype(np.float32)
    t256 = (np.arange(2 * P)[None, :] >= P + a).astype(np.float32)
    trib = np.concatenate([t128, t256], axis=1).astype(NP_BF16)

    # stt bias slabs: [bias48 | j0(SC) | j1(SC) | j2(2P) | j3(2P)]
    NEGB = -1000.0
    b48 = np.full((P, SC), EB, np.float32)
    cols = np.arange(SC)[None, :]
    bj0 = np.where(cols >= a, EB, NEGB) * (cols < P) + EB * (cols >= P)
    bj0 = np.where(cols < P, np.where(cols >= a, EB, NEGB), EB)
    bj1 = np.where(cols < 2 * P, np.where(cols >= P + a, EB, NEGB), EB)
    c2 = np.arange(2 * P)[None, :]
    bj2 = np.where(c2 < P, np.where(c2 >= a, EB, NEGB), EB)
    bj3 = np.where(c2 >= P + a, EB, NEGB)
    bias = np.concatenate([b48, bj0, bj1, bj2, bj3], axis=1).astype(NP_BF16)

    # per-batch x packs
    xbf, x8 = [], []
    for b in range(B):
        xT = np.ascontiguousarray(x[b].T)               # [D, S]
        xbf.append(np.ascontiguousarray(xT[:, :SC]).astype(NP_BF16))
        xr = (xT[:, SC:].reshape(NDT, P, 3 * SC)
              .transpose(1, 0, 2).reshape(P, NDT * 3 * SC))
        x8.append(np.ascontiguousarray(xr).astype(NP_F8E4))

    # per-group weight packs
    packs = []
    for g in range(NG):
        qc0, kc0 = g * NH * HD, g * HD
        wqg = wq[:, qc0:qc0 + NH * HD]
        wkg = wk[:, kc0:kc0 + HD]
        wvg = wv[:, kc0:kc0 + HD]
        wog = wo[qc0:qc0 + NH * HD, :]
        # wo8: head-pairs: [p, hp, i, d]
        wo8 = (wog.reshape(2, 2, P, D).transpose(2, 0, 1, 3)
               .reshape(P, 4 * D))
        packs.append({
            "wqbf": np.ascontiguousarray(wqg).astype(NP_BF16),
            "wkbf": np.ascontiguousarray(wkg).astype(NP_BF16),
            "wvbf": np.ascontiguousarray(wvg).astype(NP_BF16),
            "wobf": np.ascontiguousarray(wog).astype(NP_BF16),
            "wq8": _pack_pairs(wqg),
            "wk8": _pack_pairs(wkg),
            "wv8": _pack_pairs(wvg),
            "wo8": np.ascontiguousarray(wo8).astype(NP_F8E4),
        })

    in_maps = []
    for c in range(NCORES):
        b, g = c // NG, c % NG
        m = {
            "xbf": xbf[b], "x8": x8[b],
            "cosq": cosq, "sinq": sinq, "cosk": coskb, "sink": sinkb,
            "identb": identb, "onesb": onesb, "ones8": ones8,
            "trib": trib, "bias": bias,
            "negln2": np.full((P, 1), -np.log(2.0), np.float32),
        }
        m.update(packs[g])
        in_maps.append(m)
    return in_maps


LAST_RESULTS = None


def _make_exec(nc):
    """Mirror run_bass_via_pjrt's multi-core path, but keep the jitted
    executable so repeated (timed) dispatches skip retrace/reload."""
    import jax
    from jax.experimental.shard_map import shard_map
    from jax.sharding import Mesh, PartitionSpec

    from concourse import bass2jax, mybir as _mybir

    bass2jax.install_neuronx_cc_hook()
    partition_name = (
        nc.partition_id_tensor.name if nc.partition_id_tensor else None)
    in_names, out_names, out_avals, zero_outs = [], [], [], []
    for alloc in nc.m.functions[0].allocations:
        if not isinstance(alloc, _mybir.MemoryLocationSet):
            continue
        name = alloc.memorylocations[0].name
        if alloc.kind == "ExternalInput":
            if name != partition_name:
                in_names.append(name)
        elif alloc.kind == "ExternalOutput":
            shape = tuple(alloc.tensor_shape)
            dtype = _mybir.dt.np(alloc.dtype)
            out_names.append(name)
            out_avals.append(jax.core.ShapedArray(shape, dtype))
            zero_outs.append(np.zeros(shape, dtype))
    n_params = len(in_names)
    n_outs = len(out_avals)
    all_in_names = list(in_names) + list(out_names)
    if partition_name is not None:
        all_in_names.append(partition_name)
    donate = tuple(range(n_params, n_params + n_outs))

    def _body(*args):
        operands = list(args)
        if partition_name is not None:
            operands.append(bass2jax.partition_id_tensor())
        outs = bass2jax._bass_exec_p.bind(
            *operands,
            out_avals=tuple(out_avals),
            in_names=tuple(all_in_names),
            out_names=tuple(out_names),
            lowering_input_output_aliases=(),
            sim_require_finite=True,
            sim_require_nnan=True,
            nc=nc,
        )
        return tuple(outs)

    devices = jax.devices()[:NCORES]
    mesh = Mesh(np.asarray(devices), ("core",))
    sharded = jax.jit(
        shard_map(
            _body, mesh=mesh,
            in_specs=(PartitionSpec("core"),) * (n_params + n_outs),
            out_specs=(PartitionSpec("core"),) * n_outs,
            check_rep=False,
        ),
        donate_argnums=donate, keep_unused=True,
    )
    return {
        "fn": sharded, "in_names": in_names, "out_names": out_names,
        "out_avals": out_avals, "zero_outs": zero_outs, "mesh": mesh,
    }


def get_exec(repeat=1):
    key = ("exec", repeat, ACT_NTH)
    if key not in _CACHE:
        _CACHE[key] = _make_exec(get_program(repeat))
    return _CACHE[key]


def _concat_inputs(ex, in_maps):
    return [
        np.concatenate([np.asarray(in_maps[c][name]) for c in range(NCORES)],
                       axis=0)
        for name in ex["in_names"]
    ]


def _concat_zeros(ex):
    return [
        np.zeros((NCORES * z.shape[0], *z.shape[1:]), z.dtype)
        for z in ex["zero_outs"]
    ]


def run_on_device(in_maps, repeat=1):
    """One dispatch; returns per-core output dicts (numpy)."""
    ex = get_exec(repeat)
    out_arrs = ex["fn"](*_concat_inputs(ex, in_maps), *_concat_zeros(ex))
    res = []
    for c in range(NCORES):
        res.append({
            name: np.asarray(out_arrs[i]).reshape(
                NCORES, *ex["out_avals"][i].shape)[c]
            for i, name in enumerate(ex["out_names"])
        })
    return res


def bench(in_maps, iters=5, repeat=1):
    """Timed repeated dispatch: inputs pre-placed on device, fresh donated
    zero output buffers pre-placed per iteration. Returns list of wall ns."""
    import time

    import jax
    from jax.sharding import NamedSharding, PartitionSpec

    ex = get_exec(repeat)
    sh = NamedSharding(ex["mesh"], PartitionSpec("core"))
    dev_in = [jax.device_put(a, sh) for a in _concat_inputs(ex, in_maps)]
    zsets = [[jax.device_put(z, sh) for z in _concat_zeros(ex)]
             for _ in range(iters + 1)]
    jax.block_until_ready(dev_in)
    jax.block_until_ready(zsets)
    out = ex["fn"](*dev_in, *zsets[0])       # warm-up
    jax.block_until_ready(out)
    times = []
    for i in range(iters):
        t0 = time.perf_counter()
        out = ex["fn"](*dev_in, *zsets[i + 1])
        jax.block_until_ready(out)
        times.append((time.perf_counter() - t0) * 1e9)
    return times


def bench_slope(in_maps, iters=8, r_hi=4):
    """Per-iteration kernel time via slope: (T(r_hi) - T(1)) / (r_hi - 1).
    Immune to constant dispatch overhead."""
    t1 = bench(in_maps, iters=iters, repeat=1)
    th = bench(in_maps, iters=iters, repeat=r_hi)
    t1m, thm = np.median(t1), np.median(th)
    t1b, thb = min(t1), min(th)
    return {
        "t1": t1, "th": th,
        "exec_ns_median": (thm - t1m) / (r_hi - 1),
        "exec_ns_min": (thb - t1b) / (r_hi - 1),
    }


def kernel(x, wq, wk, wv, wo, mask):
    """Full inputs in, full output out; shards over the 8 NeuronCores."""
    global LAST_RESULTS
    from concourse import bass_utils

    nc = get_program()
    in_maps = make_in_maps(x, wq, wk, wv, wo, mask)
    res = bass_utils.run_bass_kernel_spmd(
        nc, in_maps, core_ids=list(range(NCORES)))
    LAST_RESULTS = res
    out = np.zeros((B, S, D), dtype=np.float32)
    for c in range(NCORES):
        b = c // NG
        out[b] += res.results[c]["y"]
    return out


# revision 11
# speedup vs baseline: 3.7471x; 2.7938x over previous
"""Trainium2 Bass kernel for GQA attention (B=2,S=2048,D=2048,H=16,KV=4,HD=128)
with RoPE + causal mask, sharded over 8 NeuronCores:
  2-way data parallel over batch x 4-way tensor parallel over KV groups.

Core c = (b, g): b = c // 4, g = c % 4.
Each core computes, for its batch b and KV group g (q heads 4g..4g+3):
  QT_h [HD,S], KT [HD,S] (RoPE'd), V [S,HD]    via matmul vs xT [D,S]
  scoresT [sk,sq] blocks, exp on ScalarE (scale folded), row-sums via an
  all-ones matmul (which also replicates the sums across partitions),
  AV with V tiles stationary -> outT [HD,sq], per-head normalization via
  reciprocal, partial y = attn_norm @ wo_rows[g]; host sums the 4 partials.

matmul(out, lhsT, rhs) = lhsT.T @ rhs, contraction over the partition dim.
All contractions are K=128.  Causality at block granularity: fully-masked
(sk,sq) blocks skipped; diagonal blocks add the mask slice (pattern repeats
every 4 sk-tiles, so only a [512,512] mask transpose is shipped).
"""

import os
from contextlib import ExitStack

import numpy as np

import concourse.bacc as bacc
import concourse.mybir as mybir
import concourse.tile as tile

# ---------------- problem constants (hardcoded per contract) ----------------
B, S, D = 2, 2048, 2048
H, KV, HD = 16, 4, 128
REP = H // KV            # 4 q heads per kv head
NG = KV                  # 4 tensor-parallel groups
NCORES = 8
THETA = 10000.0
SCALE = 1.0 / float(np.sqrt(HD))

P = 128                  # partition dim
SC = 512                 # moving free-dim chunk (fp32 max)
NDT = S // P             # 16 tiles of 128 along S or D
NCH = S // SC            # 4 chunks of 512 along S
NH = REP                 # 4 q-heads per core

FP32 = mybir.dt.float32
F32R = mybir.dt.float32r

# matmul dtype: "f32r" (tf32-class operands, 1 cyc/row, measured full-kernel
# relative error ~2.2e-4 vs the fp32 reference) or "fp32" (exact to ~1.4e-6,
# but 4 cyc/row on the PE).  The attention output is a softmax-weighted
# average followed by modest-depth sums, so tf32-class operand rounding
# stays well inside a scale-relative tolerance.
MM_MODE = os.environ.get("KERNEL_MM_MODE", "f32r")

_CACHE = {}


def _build_program(mm_mode=MM_MODE, repeat=1):
    # MDT: dtype of every matmul operand tile (and the DRAM tensors DMA'd
    # straight into them -- the BIR verifier requires fp32r matmult inputs
    # to be *produced* as fp32r).
    MDT = F32R if mm_mode == "f32r" else FP32

    nc = bacc.Bacc("TRN2", target_bir_lowering=False, debug=False)

    xT_d = nc.dram_tensor("xT", [D, S], MDT, kind="ExternalInput").ap()
    wq_d = nc.dram_tensor("wqg", [D, NH * HD], MDT, kind="ExternalInput").ap()
    wk_d = nc.dram_tensor("wkg", [D, HD], MDT, kind="ExternalInput").ap()
    wv_d = nc.dram_tensor("wvg", [D, HD], MDT, kind="ExternalInput").ap()
    wo_d = nc.dram_tensor("wog", [NH * HD, D], MDT, kind="ExternalInput").ap()
    cosT_d = nc.dram_tensor("cosT", [HD, S], FP32, kind="ExternalInput").ap()
    sinrT_d = nc.dram_tensor("sinrotT", [HD, S], FP32, kind="ExternalInput").ap()
    mdiag_d = nc.dram_tensor("maskdiag", [NCH * P, SC], FP32, kind="ExternalInput").ap()
    ident_d = nc.dram_tensor("ident", [P, P], FP32, kind="ExternalInput").ap()
    ones_d = nc.dram_tensor("ones", [P, P], MDT, kind="ExternalInput").ap()
    y_d = nc.dram_tensor("y", [S, D], FP32, kind="ExternalOutput").ap()

    with tile.TileContext(nc) as tc, ExitStack() as ctx:
        qkv = ctx.enter_context(tc.tile_pool(name="qkv", bufs=1))
        small = ctx.enter_context(tc.tile_pool(name="small", bufs=1))

        # resident Q^T per head, K^T, V tiles
        qt = [qkv.tile([P, S], MDT, tag=f"qt{h}", name=f"qt{h}") for h in range(NH)]
        kt = qkv.tile([P, S], MDT, tag="kt", name="kt")
        v_tiles = [qkv.tile([P, HD], MDT, tag=f"v{k}", name=f"v{k}")
                   for k in range(NDT)]

        ones_sb = small.tile([P, P], MDT, tag="ones")
        ident_sb = small.tile([P, P], FP32, tag="ident")
        mdiag_slab = small.tile([P, NCH * SC], FP32, tag="mds")
        mdiag_sb = [mdiag_slab[:, r * SC:(r + 1) * SC] for r in range(NCH)]

        def load_consts():
            nc.gpsimd.dma_start(ones_sb[:], ones_d[:])
            nc.sync.dma_start(ident_sb[:], ident_d[:])
            nc.gpsimd.dma_start(
                mdiag_slab[:].rearrange("p (r s) -> p r s", r=NCH),
                mdiag_d.rearrange("(r p) s -> p r s", p=P))

        for rep in range(repeat):
            # ============== phase 1: QKV projection + RoPE ==============
            with tc.tile_pool(name="p1", bufs=1) as p1, \
                 tc.tile_pool(name="xin", bufs=6) as xin, \
                 tc.tile_pool(name="rtmp", bufs=3) as rtmp, \
                 tc.tile_pool(name="ps1", bufs=2, space="PSUM") as ps1:

                # weight slab: tile k of wq lives at slab columns
                # [k*512, (k+1)*512), head slice m at [k*512 + m*128, ...).
                # Loads are split into quarters and spread over two DMA
                # queues (sync + scalar) so the first matmul chain is gated
                # by ~2MB, not the full 13MB of phase-1 inputs.
                XQ = NDT // 4   # 4 d-tiles per quarter slab
                wqs = p1.tile([P, NDT * NH * HD], MDT, tag="wqs")
                wks = p1.tile([P, NDT * HD], MDT, tag="wks")
                wvs = p1.tile([P, NDT * HD], MDT, tag="wvs")
                cosT_sb = p1.tile([HD, S], FP32, tag="cosT")
                sinrT_sb = p1.tile([HD, S], FP32, tag="sinrT")
                for qq in range(4):
                    r0, r1 = qq * XQ * P, (qq + 1) * XQ * P
                    nc.scalar.dma_start(
                        wqs[:, qq * XQ * NH * HD:(qq + 1) * XQ * NH * HD]
                        .rearrange("p (n m) -> p n m", n=XQ),
                        wq_d[r0:r1, :].rearrange("(n p) m -> p n m", p=P))
                vT = p1.tile([HD, S], FP32, tag="vT")

                for sc in range(NCH):
                    s0 = sc * SC
                    xq_slabs = []
                    for qq in range(4):
                        xs = xin.tile([P, XQ * SC], MDT, tag="x")
                        nc.sync.dma_start(
                            xs[:].rearrange("p (n s) -> p n s", n=XQ),
                            xT_d[qq * XQ * P:(qq + 1) * XQ * P, s0:s0 + SC]
                            .rearrange("(n p) s -> p n s", p=P))
                        xq_slabs.append(xs)
                    if sc == 0:
                        # needed only from the first RoPE / m=4 onwards;
                        # issue after chunk-0 x so the first chain starts asap
                        nc.gpsimd.dma_start(cosT_sb[:], cosT_d[:])
                        nc.gpsimd.dma_start(sinrT_sb[:], sinrT_d[:])
                        nc.gpsimd.dma_start(
                            wks[:].rearrange("p (n m) -> p n m", n=NDT),
                            wk_d.rearrange("(n p) m -> p n m", p=P))
                        nc.gpsimd.dma_start(
                            wvs[:].rearrange("p (n m) -> p n m", n=NDT),
                            wv_d.rearrange("(n p) m -> p n m", p=P))
                        if rep == 0:
                            load_consts()

                    def xts_k(k):
                        return xq_slabs[k // XQ][:, (k % XQ) * SC:(k % XQ + 1) * SC]

                    # m = 0..3: q heads; 4: k; 5: v
                    for m in range(NH + 2):
                        psum = ps1.tile([P, SC], FP32, tag="proj", bufs=4)
                        for k in range(NDT):
                            if m < NH:
                                lhsT = wqs[:, k * NH * HD + m * HD:
                                           k * NH * HD + (m + 1) * HD]
                            elif m == NH:
                                lhsT = wks[:, k * HD:(k + 1) * HD]
                            else:
                                lhsT = wvs[:, k * HD:(k + 1) * HD]
                            nc.tensor.matmul(
                                psum[:], lhsT, xts_k(k),
                                start=(k == 0), stop=(k == NDT - 1),
                            )
                        if m <= NH:
                            # RoPE: dst = psum*cosT + shift(psum)*sinrotT
                            dst = (qt[m] if m < NH else kt)[:, s0:s0 + SC]
                            t0 = rtmp.tile([P, SC], FP32, tag="t0")
                            t1 = rtmp.tile([P, SC], FP32, tag="t1")
                            nc.vector.tensor_mul(
                                t0[:], psum[:], cosT_sb[:, s0:s0 + SC])
                            nc.vector.tensor_mul(
                                t1[0:64, :], psum[64:128, :],
                                sinrT_sb[0:64, s0:s0 + SC])
                            nc.vector.tensor_mul(
                                t1[64:128, :], psum[0:64, :],
                                sinrT_sb[64:128, s0:s0 + SC])
                            nc.vector.tensor_add(dst, t0[:], t1[:])
                        else:
                            nc.vector.tensor_copy(vT[:, s0:s0 + SC], psum[:])

                    # transpose this chunk of V^T -> V tiles [S_k=128, HD]
                    for kk in range(SC // P):
                        k = sc * (SC // P) + kk
                        ps_t = ps1.tile([P, P], FP32, tag="vt")
                        nc.tensor.transpose(
                            ps_t[:], vT[:, k * P:(k + 1) * P], ident_sb[:])
                        nc.vector.tensor_copy(v_tiles[k][:], ps_t[:])

            # ========== phase 2: attention + output projection ==========
            with tc.tile_pool(name="p2", bufs=1) as p2, \
                 tc.tile_pool(name="pt", bufs=24) as ptp, \
                 tc.tile_pool(name="nrm", bufs=4) as nrm, \
                 tc.tile_pool(name="yst", bufs=3) as yst, \
                 tc.tile_pool(name="ps2", bufs=2, space="PSUM") as ps2, \
                 tc.tile_pool(name="pss", bufs=2, space="PSUM") as pss:

                wos = p2.tile([P, NH * D], MDT, tag="wos")
                nc.sync.dma_start(
                    wos[:].rearrange("p (n d) -> p n d", n=NH),
                    wo_d.rearrange("(n p) d -> p n d", p=P))
                wo_sb = [wos[:, h * D:(h + 1) * D] for h in range(NH)]
                outT = [p2.tile([P, SC], MDT, tag=f"ot{h}", name=f"ot{h}")
                        for h in range(NH)]

                for c in range(NCH):
                    q0 = c * SC
                    nk = 4 * c + 4          # active sk tiles (causal)
                    for h in range(NH):
                        pts = []
                        offs = []
                        # all-ones stationary -> every psum partition gets
                        # the column sum over sk (broadcast for free)
                        sums_ps = pss.tile([P, SC], FP32, tag="sums", bufs=1)
                        for k in range(nk):
                            # diagonal blocks: sk tile k only attends to
                            # sq >= 128k, i.e. chunk columns [off:512).
                            # f32r matmuls need moving dim >= 256 for the
                            # 1 cyc/row mode, so keep at least 256 columns
                            # (the extra columns are masked -> exp -> 0).
                            off = max(0, (k - 4 * c) * P)
                            if MDT == F32R:
                                off = min(off, SC - 2 * P)
                            sc_ps = ps2.tile([P, SC], FP32, tag="sc", bufs=3)
                            nc.tensor.matmul(
                                sc_ps[:, off:],
                                kt[:, k * P:(k + 1) * P],
                                qt[h][:, q0 + off:q0 + SC],
                                start=True, stop=True,
                            )
                            pt = ptp.tile([P, SC], MDT, tag="pt")
                            if k >= 4 * c:
                                # diagonal block: scores*scale + mask, exp
                                r = k % NCH
                                nc.vector.scalar_tensor_tensor(
                                    sc_ps[:, off:], sc_ps[:, off:], SCALE,
                                    mdiag_sb[r][:, off:],
                                    op0=mybir.AluOpType.mult,
                                    op1=mybir.AluOpType.add)
                                nc.scalar.activation(
                                    pt[:, off:], sc_ps[:, off:],
                                    mybir.ActivationFunctionType.Exp)
                            else:
                                nc.scalar.activation(
                                    pt[:, off:], sc_ps[:, off:],
                                    mybir.ActivationFunctionType.Exp,
                                    scale=SCALE)
                            pts.append(pt)
                            offs.append(off)
                        for k in range(nk):
                            nc.tensor.matmul(
                                sums_ps[:, offs[k]:], ones_sb[:],
                                pts[k][:, offs[k]:],
                                start=(k == 0), stop=(k == nk - 1),
                            )
                        # AV: outT_h [HD, sq] = sum_k V_k^T @ probsT_k
                        av_ps = ps2.tile([P, SC], FP32, tag="av")
                        for k in range(nk):
                            nc.tensor.matmul(
                                av_ps[:, offs[k]:], v_tiles[k][:],
                                pts[k][:, offs[k]:],
                                start=(k == 0), stop=(k == nk - 1),
                            )
                        # normalize: outT[h] = av * (1/sums)
                        recip = nrm.tile([P, SC], FP32, tag="recip")
                        nc.vector.reciprocal(recip[:], sums_ps[:])
                        nc.vector.tensor_mul(outT[h][:], av_ps[:], recip[:])

                    # output projection for this sq chunk; results are
                    # staged in half-slabs (t pairs) and stored with one
                    # batched DMA each on the otherwise-idle gpsimd queue
                    for t in range(SC // P):
                        yslab = yst.tile([P, D], FP32, tag="yslab")
                        for dci in range(NCH):
                            d0 = dci * SC
                            y_ps = ps2.tile([P, SC], FP32, tag="y", bufs=2)
                            for h in range(NH):
                                nc.tensor.matmul(
                                    y_ps[:],
                                    outT[h][:, t * P:(t + 1) * P],
                                    wo_sb[h][:, d0:d0 + SC],
                                    start=(h == 0), stop=(h == NH - 1),
                                )
                            nc.vector.tensor_copy(
                                yslab[:, d0:d0 + SC], y_ps[:])
                        row0 = q0 + t * P
                        nc.gpsimd.dma_start(
                            y_d[row0:row0 + P, :], yslab[:])

    nc.compile()
    return nc


def _host_tables():
    inv_freq = 1.0 / (THETA ** (np.arange(0, HD, 2, dtype=np.float32) / HD))
    t = np.arange(S, dtype=np.float32)
    freqs = t[:, None] * inv_freq[None, :]              # [S, HD/2]
    emb = np.concatenate([freqs, freqs], axis=-1)       # [S, HD]
    cos = np.cos(emb).astype(np.float32)
    sin = np.sin(emb).astype(np.float32)
    cosT = np.ascontiguousarray(cos.T)                  # [HD, S]
    sinT = np.ascontiguousarray(sin.T)
    sinrotT = sinT.copy()
    sinrotT[0:HD // 2] = -sinT[0:HD // 2]
    return cosT, sinrotT


def get_program(mm_mode=MM_MODE, repeat=1):
    key = ("nc", mm_mode, repeat)
    if key not in _CACHE:
        _CACHE[key] = _build_program(mm_mode, repeat)
    return _CACHE[key]


def make_in_maps(x, wq, wk, wv, wo, mask):
    x = np.asarray(x, dtype=np.float32)
    wq = np.asarray(wq, dtype=np.float32)
    wk = np.asarray(wk, dtype=np.float32)
    wv = np.asarray(wv, dtype=np.float32)
    wo = np.asarray(wo, dtype=np.float32)
    mask = np.asarray(mask, dtype=np.float32)

    cosT, sinrotT = _host_tables()
    ident = np.eye(P, dtype=np.float32)
    # maskdiag[r*128+a, b] = mask[0,0, b, r*128+a]; pattern repeats per chunk
    maskdiag = np.ascontiguousarray(mask[0, 0, 0:SC, 0:SC].T)

    xT = [np.ascontiguousarray(x[b].T) for b in range(B)]
    in_maps = []
    for c in range(NCORES):
        b, g = c // NG, c % NG
        qc0 = g * NH * HD
        kc0 = g * HD
        in_maps.append({
            "xT": xT[b],
            "wqg": np.ascontiguousarray(wq[:, qc0:qc0 + NH * HD]),
            "wkg": np.ascontiguousarray(wk[:, kc0:kc0 + HD]),
            "wvg": np.ascontiguousarray(wv[:, kc0:kc0 + HD]),
            "wog": np.ascontiguousarray(wo[qc0:qc0 + NH * HD, :]),
            "cosT": cosT,
            "sinrotT": sinrotT,
            "maskdiag": maskdiag,
            "ident": ident,
            "ones": np.ones((P, P), dtype=np.float32),
        })
    return in_maps


LAST_RESULTS = None


def _make_exec(nc):
    """Mirror run_bass_via_pjrt's multi-core path, but keep the jitted
    executable so repeated (timed) dispatches skip retrace/reload."""
    import jax
    from jax.experimental.shard_map import shard_map
    from jax.sharding import Mesh, PartitionSpec

    from concourse import bass2jax, mybir as _mybir

    bass2jax.install_neuronx_cc_hook()
    partition_name = (
        nc.partition_id_tensor.name if nc.partition_id_tensor else None)
    in_names, out_names, out_avals, zero_outs = [], [], [], []
    for alloc in nc.m.functions[0].allocations:
        if not isinstance(alloc, _mybir.MemoryLocationSet):
            continue
        name = alloc.memorylocations[0].name
        if alloc.kind == "ExternalInput":
            if name != partition_name:
                in_names.append(name)
        elif alloc.kind == "ExternalOutput":
            shape = tuple(alloc.tensor_shape)
            dtype = _mybir.dt.np(alloc.dtype)
            out_names.append(name)
            out_avals.append(jax.core.ShapedArray(shape, dtype))
            zero_outs.append(np.zeros(shape, dtype))
    n_params = len(in_names)
    n_outs = len(out_avals)
    all_in_names = list(in_names) + list(out_names)
    if partition_name is not None:
        all_in_names.append(partition_name)
    donate = tuple(range(n_params, n_params + n_outs))

    def _body(*args):
        operands = list(args)
        if partition_name is not None:
            operands.append(bass2jax.partition_id_tensor())
        outs = bass2jax._bass_exec_p.bind(
            *operands,
            out_avals=tuple(out_avals),
            in_names=tuple(all_in_names),
            out_names=tuple(out_names),
            lowering_input_output_aliases=(),
            sim_require_finite=True,
            sim_require_nnan=True,
            nc=nc,
        )
        return tuple(outs)

    devices = jax.devices()[:NCORES]
    mesh = Mesh(np.asarray(devices), ("core",))
    sharded = jax.jit(
        shard_map(
            _body, mesh=mesh,
            in_specs=(PartitionSpec("core"),) * (n_params + n_outs),
            out_specs=(PartitionSpec("core"),) * n_outs,
            check_rep=False,
        ),
        donate_argnums=donate, keep_unused=True,
    )
    return {
        "fn": sharded, "in_names": in_names, "out_names": out_names,
        "out_avals": out_avals, "zero_outs": zero_outs, "mesh": mesh,
    }


def get_exec(mm_mode=MM_MODE, repeat=1):
    key = ("exec", mm_mode, repeat)
    if key not in _CACHE:
        _CACHE[key] = _make_exec(get_program(mm_mode, repeat))
    return _CACHE[key]


def _concat_inputs(ex, in_maps):
    return [
        np.concatenate([np.asarray(in_maps[c][name]) for c in range(NCORES)],
                       axis=0)
        for name in ex["in_names"]
    ]


def _concat_zeros(ex):
    return [
        np.zeros((NCORES * z.shape[0], *z.shape[1:]), z.dtype)
        for z in ex["zero_outs"]
    ]


def run_on_device(in_maps, mm_mode=MM_MODE, repeat=1):
    """One dispatch; returns per-core output dicts (numpy)."""
    ex = get_exec(mm_mode, repeat)
    out_arrs = ex["fn"](*_concat_inputs(ex, in_maps), *_concat_zeros(ex))
    res = []
    for c in range(NCORES):
        res.append({
            name: np.asarray(out_arrs[i]).reshape(
                NCORES, *ex["out_avals"][i].shape)[c]
            for i, name in enumerate(ex["out_names"])
        })
    return res


def bench(in_maps, iters=5, mm_mode=MM_MODE, repeat=1):
    """Timed repeated dispatch: inputs pre-placed on device, fresh donated
    zero output buffers pre-placed per iteration. Returns list of wall ns."""
    import time

    import jax
    from jax.sharding import NamedSharding, PartitionSpec

    ex = get_exec(mm_mode, repeat)
    sh = NamedSharding(ex["mesh"], PartitionSpec("core"))
    dev_in = [jax.device_put(a, sh) for a in _concat_inputs(ex, in_maps)]
    zsets = [[jax.device_put(z, sh) for z in _concat_zeros(ex)]
             for _ in range(iters + 1)]
    jax.block_until_ready(dev_in)
    jax.block_until_ready(zsets)
    out = ex["fn"](*dev_in, *zsets[0])       # warm-up
    jax.block_until_ready(out)
    times = []
    for i in range(iters):
        t0 = time.perf_counter()
        out = ex["fn"](*dev_in, *zsets[i + 1])
        jax.block_until_ready(out)
        times.append((time.perf_counter() - t0) * 1e9)
    return times


def bench_slope(in_maps, iters=8, mm_mode=MM_MODE, r_hi=4):
    """Per-iteration kernel time via slope: (T(r_hi) - T(1)) / (r_hi - 1).
    Immune to constant dispatch overhead."""
    t1 = bench(in_maps, iters=iters, mm_mode=mm_mode, repeat=1)
    th = bench(in_maps, iters=iters, mm_mode=mm_mode, repeat=r_hi)
    t1m, thm = np.median(t1), np.median(th)
    t1b, thb = min(t1), min(th)
    return {
        "t1": t1, "th": th,
        "exec_ns_median": (thm - t1m) / (r_hi - 1),
        "exec_ns_min": (thb - t1b) / (r_hi - 1),
    }


def kernel(x, wq, wk, wv, wo, mask):
    """Full inputs in, full output out; shards over the 8 NeuronCores."""
    global LAST_RESULTS
    from concourse import bass_utils

    nc = get_program()
    in_maps = make_in_maps(x, wq, wk, wv, wo, mask)
    res = bass_utils.run_bass_kernel_spmd(
        nc, in_maps, core_ids=list(range(NCORES)))
    LAST_RESULTS = res
    out = np.zeros((B, S, D), dtype=np.float32)
    for c in range(NCORES):
        b = c // NG
        out[b] += res.results[c]["y"]
    return out

